# revision 56
# baseline (speedup 1.0000x reference)
"""Trainium2 Bass kernel for nn_EosLayer (gated linear-attention recurrence).

Sharding: 8 cores = 4 batches x 2 sequence halves. Each core processes
T = 256 (warmup) + 2048 (output) timesteps of one batch half. The warmup
window replaces cross-core state passing: the per-(k,d) decay o < 0.97
makes history older than 256 steps contribute < 1e-3 relative.

Per-core layout is d-major (d on partitions, time on the free dim).
The hardware prefix scan (TensorScalarPtr, DVE-only on TRN2) runs in one
instruction per (chunk, d-tile): the 8 k-slots are fused into a single
scan via an on-device decay pattern with zeroed "break" columns, into
which the previous chunk's carries are injected; the scan state is fp32
internally regardless of operand dtype. The remaining elementwise work
(z = e*i, s*m, k-reduction tree) runs in bf16 (DVE 2x_1p fast mode),
with the z-multiplies offloaded to the Pool/GpSimd engine so the DVE
stays scan+y-path bound. LayerNorm stats come out of the tensor engine
in column form (free-size-1 matmuls), so the LN scalar math is [128, 4]-
shaped and no rsig row->column transpose is needed; gamma/beta are
folded into W_out on the host (the beta term is skipped when it is
exactly zero). The chunk loop is software-pipelined: projections for
chunk c+1 and the y-path for chunk c-1 overlap the scan of chunk c, and
the final chunk's y-path is issued in quarter-width blocks to overlap
its epilogue chain.
"""

import numpy as np
import ml_dtypes

D = 512
K = 8
TAU = 16.0
EPS = 1e-5
B = 4
N = 4096
H = N // 2          # output rows per core
W = 256             # warmup rows
T = W + H           # 2304 rows processed per core
TC = 512            # chunk length (free-dim columns per chunk)
SEG = TC + 1        # k-run length in the scan buffer (break col + TC)
NCHUNK = 5          # chunk 0 is the (256-col) warmup chunk
NDO = 4             # d-tiles of 128 partitions
P = 128
NTT = TC // P       # 4 t-blocks per chunk in the epilogue

_CACHE = {}

# engine assignment knobs (per d-tile, True = Pool/gpsimd, False = DVE).
# TensorScalarPtr (scan, scalar_tensor_tensor) is DVE-only on real HW;
# TensorTensor runs on either engine (Pool at 0.42 efficiency).
Z_POOL = (True, True, True, True)       # z = e*i, chunks 2+
Z1_POOL = (True, False, False, False)   # z for chunk 1 (pipeline ramp)
ZW_POOL = (False, False, True, True)    # warm-chunk z
DPAT_ON_DEVICE = True    # build the decay pattern with Act instead of DMA
SMUL_POOL = (False, False, False, False)
L1_POOL = (False, False, False, False)
L2_POOL = (False, False, False, False)
L3_POOL = (False, False, False, False)
EPI_ACT_POOL = True   # True: Act scale-evac + Pool f32 add; False: DVE stt
Y_BEFORE_SCAN = True   # issue y(c-1) DVE ops before scan(c) in the loop
HAS_BOW = True        # False when beta @ W_out == 0: skip the epilogue add


def _build():
    import concourse.bass as bass
    import concourse.mybir as mybir
    import concourse.tile as tile
    from concourse.bacc import Bacc

    f32 = mybir.dt.float32
    bf16 = mybir.dt.bfloat16
    AF = mybir.ActivationFunctionType
    OP = mybir.AluOpType

    nc = Bacc("TRN2", target_bir_lowering=False, debug=False,
              enable_asserts=False, num_devices=8)

    xt = nc.dram_tensor("xt", (D, T), bf16, kind="ExternalInput")
    wi = nc.dram_tensor("wi", (D, D), bf16, kind="ExternalInput")
    wes = nc.dram_tensor("wes", (D, 2 * K), bf16, kind="ExternalInput")
    oc = nc.dram_tensor("oc", (D, K), f32, kind="ExternalInput")       # o.T
    dpa = None
    if not DPAT_ON_DEVICE:
        dpa = nc.dram_tensor("dpa", (D, K * SEG), f32, kind="ExternalInput")
    wo = nc.dram_tensor("wo", (D, D), bf16, kind="ExternalInput")      # gamma-folded
    hrow = nc.dram_tensor("hrow", (1, D), bf16, kind="ExternalInput")  # colsum(wo)
    bowr = nc.dram_tensor("bowr", (1, D), f32, kind="ExternalInput")   # beta @ W_out
    yout = nc.dram_tensor("yout", (H, D), f32, kind="ExternalOutput")

    def strided(ap, off, dims):
        return bass.AP(tensor=ap.tensor, offset=ap.offset + off,
                       ap=[ap.ap[0]] + dims)

    with tile.TileContext(nc) as tc:
        with tc.tile_pool(name="const", bufs=1) as cst, \
             tc.tile_pool(name="wk", bufs=1) as wk, \
             tc.tile_pool(name="db", bufs=2) as db, \
             tc.tile_pool(name="zmp", bufs=2) as zmp, \
             tc.tile_pool(name="pit", bufs=3, space="PSUM") as pit, \
             tc.tile_pool(name="pes", bufs=1, space="PSUM") as pes, \
             tc.tile_pool(name="pg", bufs=2, space="PSUM") as pg, \
             tc.tile_pool(name="ptc", bufs=1, space="PSUM") as ptc, \
             tc.tile_pool(name="pmr", bufs=1, space="PSUM") as pmr, \
             tc.tile_pool(name="dr", bufs=2, space="DRAM") as dr:

            # ---- constants (loaded once) ----
            wi_all = cst.tile([P, NDO * D], bf16, tag="wi", name="wi")
            wi_sb = [wi_all[:, t * D:(t + 1) * D] for t in range(NDO)]
            wes_all = cst.tile([P, NDO * 2 * K], bf16, tag="wes", name="wes")
            wes_sb = [wes_all[:, t * 2 * K:(t + 1) * 2 * K]
                      for t in range(NDO)]
            oc_all = cst.tile([P, NDO * K], f32, tag="oc", name="oc")
            oc_sb = [oc_all[:, t * K:(t + 1) * K] for t in range(NDO)]
            dp_sb = [cst.tile([P, K * SEG], f32, tag=f"dp{t}", name=f"dp{t}")
                     for t in range(NDO)]
            wo_all = cst.tile([P, NDO * D], bf16, tag="wo", name="wo")
            wo_sb = [wo_all[:, t * D:(t + 1) * D] for t in range(NDO)]
            h_sb = cst.tile([1, D], bf16, tag="h", name="h")
            bow_rep = (cst.tile([P, D], f32, tag="bow", name="bow")
                       if HAS_BOW else None)
            ones_sb = cst.tile([P, 1], bf16, tag="ones", name="ones")
            nc.vector.memset(ones_sb, 1.0)
            eps_col = cst.tile([P, 1], f32, tag="eps", name="eps")
            nc.vector.memset(eps_col, EPS)

            def late_consts():
                # issued after chunk 0's input chain: dpat is first needed
                # by scan(1), wo/h/bow by the chunk-1 epilogue
                if DPAT_ON_DEVICE:
                    # dpat[d, k*SEG] = 0; dpat[d, k*SEG+1..] = oc[d, k]
                    for t in range(NDO):
                        brk = strided(dp_sb[t], 0, [[SEG, K]])
                        nc.gpsimd.memset(brk, 0.0)
                        for k in range(K):
                            nc.scalar.activation(
                                out=dp_sb[t][:, k * SEG + 1:(k + 1) * SEG],
                                in_=wi_sb[t][:, 0:TC], func=AF.Identity,
                                scale=0.0, bias=oc_sb[t][:, k:k + 1])
                else:
                    for t in range(NDO):
                        sl = slice(t * P, (t + 1) * P)
                        nc.sync.dma_start(out=dp_sb[t], in_=dpa[sl, :])
                wosrc = bass.AP(tensor=wo, offset=0,
                                ap=[[D, P], [P * D, NDO], [1, D]])
                nc.sync.dma_start(out=strided(wo_all, 0, [[D, NDO], [1, D]]),
                                  in_=wosrc)
                nc.sync.dma_start(out=h_sb, in_=hrow[:, :])
                if HAS_BOW:
                    bsrc = bass.AP(tensor=bowr, offset=0, ap=[[0, P], [1, D]])
                    nc.sync.dma_start(out=bow_rep, in_=bsrc)

            # ---- per-chunk working tiles (ping-pong via tags) ----
            def cw(c):          # column width of chunk c
                return W if c == 0 else TC

            def c0(c):          # first xt column of chunk c
                return 0 if c == 0 else W + (c - 1) * TC

            xt_sb = {}
            it_sb = {}
            es_sb = {}
            es_d = {}
            e_rep = {}
            s_rep = {}
            zm = {}
            y2t = {}

            def ensure_zm(c):
                if c not in zm:
                    zm[c] = [zmp.tile([P, K * SEG], bf16, tag=f"zm{t}",
                                      name=f"zm{t}") for t in range(NDO)]

            def stage_xt(c):
                w = cw(c)
                xta = db.tile([P, NDO * TC], bf16, tag="xta", name="xta")
                xt_sb[c] = [xta[:, t * TC:(t + 1) * TC] for t in range(NDO)]
                xsrc = bass.AP(tensor=xt, offset=c0(c),
                               ap=[[T, P], [P * T, NDO], [1, w]])
                nc.sync.dma_start(out=strided(xta, 0, [[TC, NDO], [1, w]]),
                                  in_=xsrc)

            def stage_a(c):
                """Projections + e-broadcast for chunk c."""
                w = cw(c)
                if c not in xt_sb:
                    stage_xt(c)
                # e/s projection FIRST: it heads the longest chain
                # (evac -> DRAM bounce -> replicate -> z)
                es_ps = pes.tile([2 * K, TC], f32, tag="esps", name="esps")
                for kt in range(NDO):
                    nc.tensor.matmul(es_ps[:, 0:w], wes_sb[kt][:, :],
                                     xt_sb[c][kt][:, 0:w],
                                     start=(kt == 0), stop=(kt == NDO - 1))
                es_sb[c] = wk.tile([2 * K, TC], bf16, tag="es", name="es")
                nc.scalar.copy(out=es_sb[c][:, 0:w], in_=es_ps[:, 0:w])
                es_d[c] = dr.tile([2 * K, TC], bf16, tag="esd", name="esd")
                nc.sync.dma_start(out=es_d[c][:, 0:w], in_=es_sb[c][:, 0:w])
                # replicate e rows across all partitions (via DRAM bounce)
                e_rep[c] = db.tile([P, K * TC], bf16, tag="erep", name="erep")
                esrc = bass.AP(tensor=es_d[c].tensor, offset=es_d[c].offset,
                               ap=[[0, P], [TC, K], [1, w]])
                nc.sync.dma_start(out=strided(e_rep[c], 0, [[w, K], [1, w]]),
                                  in_=esrc)
                stage_ai(c)

            def stage_ai(c):
                # i projection: 4 m-tiles x 4 contraction tiles
                w = cw(c)
                it_sb[c] = [db.tile([P, TC], bf16, tag=f"it{t}", name=f"it{t}")
                            for t in range(NDO)]
                for m in range(NDO):
                    it_ps = pit.tile([P, TC], f32, tag="itps", name="itps")
                    for kt in range(NDO):
                        nc.tensor.matmul(
                            it_ps[:, 0:w],
                            wi_sb[kt][:, m * P:(m + 1) * P],
                            xt_sb[c][kt][:, 0:w],
                            start=(kt == 0), stop=(kt == NDO - 1))
                    nc.scalar.copy(out=it_sb[c][m][:, 0:w], in_=it_ps[:, 0:w])

            def stage_srep(c):
                """Replicate s rows (consumed by stage_y(c) one chunk later)."""
                s_rep[c] = db.tile([P, K * TC], bf16, tag="srep", name="srep")
                ssrc = bass.AP(tensor=es_d[c].tensor,
                               offset=es_d[c].offset + K * TC,
                               ap=[[0, P], [TC, K], [1, TC]])
                nc.sync.dma_start(out=s_rep[c], in_=ssrc)

            def stage_z(c):
                """z = e * i into the scan buffer (bf16 2x on DVE)."""
                w = cw(c)
                seg = w + (0 if c == 0 else 1)
                off = 0 if c == 0 else 1
                ensure_zm(c)
                pool_map = (ZW_POOL if c == 0 else
                            Z1_POOL if c == 1 else Z_POOL)
                for t in range(NDO):
                    zv = strided(zm[c][t], off, [[seg, K], [1, w]])
                    ev = strided(e_rep[c], 0, [[w, K], [1, w]])
                    iv = strided(it_sb[c][t], 0, [[0, K], [1, w]])
                    eng = nc.gpsimd if pool_map[t] else nc.vector
                    eng.tensor_mul(out=zv, in0=ev, in1=iv)

            def stage_scan(c):
                """Prefix scan over time, all 8 k fused (Pool)."""
                w = cw(c)
                for t in range(NDO):
                    if c == 0:
                        # per-k scans, no carry, decay broadcast from oc col
                        for k in range(K):
                            col = oc_sb[t][:, k:k + 1]
                            dec = bass.AP(tensor=col.tensor, offset=col.offset,
                                          ap=[col.ap[0], [0, w]])
                            seg = zm[c][t][:, k * w:(k + 1) * w]
                            nc.vector.tensor_tensor_scan(
                                out=seg, data0=dec, data1=seg,
                                initial=0.0, op0=OP.mult, op1=OP.add)
                    else:
                        buf = zm[c][t][:, 0:K * SEG]
                        nc.vector.tensor_tensor_scan(
                            out=buf, data0=dp_sb[t][:, :], data1=buf,
                            initial=0.0, op0=OP.mult, op1=OP.add)

            def stage_inject(c):
                """Carry (break) columns for chunk c from chunk c-1's output."""
                pw = cw(c - 1)
                pseg = pw + (0 if c == 1 else 1)
                poff = (pw - 1) if c == 1 else pw
                ensure_zm(c)
                for t in range(NDO):
                    src = strided(zm[c - 1][t], poff, [[pseg, K]])
                    dst = strided(zm[c][t], 0, [[SEG, K]])
                    nc.scalar.copy(out=dst, in_=src)

            ytile = {}

            def stage_y_tt(c, lo=0, hi=NTT):
                """s*m and k-reduction for chunk c, t-blocks [lo, hi)."""
                b0 = lo * P
                w = (hi - lo) * P
                if lo == 0:
                    ytile[c] = (
                        [wk.tile([P, 2 * TC], bf16, tag=f"y2t{t}",
                                 name=f"y2t{t}") for t in range(NDO)],
                        ptc.tile([P, 2 * NTT], f32, tag="tcol", name="tcol"),
                        pmr.tile([1, TC], f32, tag="mrow", name="mrow"),
                        wk.tile([1, TC], bf16, tag="mneg", name="mneg"),
                        wk.tile([P, NTT], f32, tag="mu2c", name="mu2c"),
                        wk.tile([P, NTT], f32, tag="varc", name="varc"),
                        wk.tile([P, NTT], f32, tag="sigc", name="sigc"),
                        wk.tile([P, NTT], f32, tag="rsig", name="rsig"),
                    )
                    y2t[c] = ytile[c][0]
                yts, tcol, mrow, mneg, mu2c, varc, sigc, rsig = ytile[c]
                for t in range(NDO):
                    mv = strided(zm[c][t], 1 + b0, [[SEG, K], [1, w]])
                    sv = strided(s_rep[c], b0, [[TC, K], [1, w]])
                    e1 = nc.gpsimd if SMUL_POOL[t] else nc.vector
                    e1.tensor_mul(out=mv, in0=mv, in1=sv)
                    lo1 = strided(zm[c][t], 1 + b0, [[SEG, K // 2], [1, w]])
                    hi1 = strided(zm[c][t], 1 + b0 + (K // 2) * SEG,
                                  [[SEG, K // 2], [1, w]])
                    e2 = nc.gpsimd if L1_POOL[t] else nc.vector
                    e2.tensor_add(out=lo1, in0=lo1, in1=hi1)
                    lo2 = strided(zm[c][t], 1 + b0, [[SEG, 2], [1, w]])
                    hi2 = strided(zm[c][t], 1 + b0 + 2 * SEG, [[SEG, 2], [1, w]])
                    e3 = nc.gpsimd if L2_POOL[t] else nc.vector
                    e3.tensor_add(out=lo2, in0=lo2, in1=hi2)
                    e4 = nc.gpsimd if L3_POOL[t] else nc.vector
                    e4.tensor_add(
                        out=yts[t][:, b0:b0 + w],
                        in0=zm[c][t][:, 1 + b0:1 + b0 + w],
                        in1=zm[c][t][:, 1 + SEG + b0:1 + SEG + b0 + w])
                    nc.scalar.activation(out=yts[t][:, TC + b0:TC + b0 + w],
                                         in_=yts[t][:, b0:b0 + w],
                                         func=AF.Square, scale=1.0)

            def stage_y_ln(c, lo=0, hi=NTT):
                """LN stats + epilogue for chunk c, t-blocks [lo, hi)."""
                b0 = lo * P
                w = (hi - lo) * P
                yts, tcol, mrow, mneg, mu2c, varc, sigc, rsig = ytile[c]
                # LN stats in column form: free-size-1 matmuls
                for tt in range(lo, hi):
                    tsl = slice(tt * P, (tt + 1) * P)
                    for t in range(NDO):
                        nc.tensor.matmul(tcol[:, tt:tt + 1],
                                         yts[t][:, tsl], ones_sb[:, :],
                                         start=(t == 0), stop=(t == NDO - 1))
                    for t in range(NDO):
                        nc.tensor.matmul(
                            tcol[:, NTT + tt:NTT + tt + 1],
                            yts[t][:, TC + tt * P:TC + (tt + 1) * P],
                            ones_sb[:, :],
                            start=(t == 0), stop=(t == NDO - 1))
                # row form of the mean (for the rank-1 G correction)
                for t in range(NDO):
                    nc.tensor.matmul(mrow[:, b0:b0 + w], ones_sb[:, :],
                                     yts[t][:, b0:b0 + w],
                                     start=(t == 0), stop=(t == NDO - 1))
                nc.scalar.activation(out=mneg[:, b0:b0 + w],
                                     in_=mrow[:, b0:b0 + w],
                                     func=AF.Copy, scale=-1.0 / D)
                # rsig columns: 1/sqrt(Q/D - (M/D)^2 + eps), all [P, hi-lo]
                nc.scalar.activation(out=mu2c[:, lo:hi], in_=tcol[:, lo:hi],
                                     func=AF.Square, scale=1.0 / D)
                nc.vector.scalar_tensor_tensor(
                    out=varc[:, lo:hi], in0=tcol[:, NTT + lo:NTT + hi],
                    scalar=1.0 / D, in1=mu2c[:, lo:hi],
                    op0=OP.mult, op1=OP.subtract)
                nc.scalar.activation(out=sigc[:, lo:hi], in_=varc[:, lo:hi],
                                     func=AF.Sqrt, bias=eps_col[:, :],
                                     scale=1.0)
                nc.vector.reciprocal(out=rsig[:, lo:hi], in_=sigc[:, lo:hi])

                # epilogue: G = y^T Wo' + mneg x h; out = G*rsig + beta@Wout
                for tt in range(lo, hi):
                    tsl = slice(tt * P, (tt + 1) * P)
                    g_ps = pg.tile([P, D], f32, tag="gps", name="gps")
                    for t in range(NDO):
                        nc.tensor.matmul(g_ps[:, :], yts[t][:, tsl],
                                         wo_sb[t][:, :],
                                         start=(t == 0), stop=False)
                    nc.tensor.matmul(g_ps[:, :], mneg[:, tsl], h_sb[:, :],
                                     start=False, stop=True)
                    out_sb = db.tile([P, D], f32, tag="outp", name="outp")
                    if not HAS_BOW:
                        nc.scalar.activation(out=out_sb[:, :], in_=g_ps[:, :],
                                             func=AF.Copy,
                                             scale=rsig[:, tt:tt + 1])
                    elif EPI_ACT_POOL:
                        gs = db.tile([P, D], f32, tag="gsc", name="gsc")
                        nc.scalar.activation(out=gs[:, :], in_=g_ps[:, :],
                                             func=AF.Copy,
                                             scale=rsig[:, tt:tt + 1])
                        nc.gpsimd.tensor_add(out=out_sb[:, :], in0=gs[:, :],
                                             in1=bow_rep[:, :])
                    else:
                        nc.vector.scalar_tensor_tensor(
                            out=out_sb[:, :], in0=g_ps[:, :],
                            scalar=rsig[:, tt:tt + 1], in1=bow_rep[:, :],
                            op0=OP.mult, op1=OP.add)
                    orow = (c - 1) * TC + tt * P
                    nc.sync.dma_start(out=yout[orow:orow + P, :],
                                      in_=out_sb[:, :])

            # ---- software-pipelined chunk loop ----
            # iter c issues: proj(c+1) | z(c) | scan(c) | y(c-1) | carries(c+1)
            # chunk-0 inputs lead the DMA queue; then the critical consts
            stage_xt(0)
            wessrc = bass.AP(tensor=wes, offset=0,
                             ap=[[2 * K, P], [P * 2 * K, NDO], [1, 2 * K]])
            nc.sync.dma_start(
                out=strided(wes_all, 0, [[2 * K, NDO], [1, 2 * K]]),
                in_=wessrc)
            wisrc = bass.AP(tensor=wi, offset=0,
                            ap=[[D, P], [P * D, NDO], [1, D]])
            nc.sync.dma_start(out=strided(wi_all, 0, [[D, NDO], [1, D]]),
                              in_=wisrc)
            ocsrc = bass.AP(tensor=oc, offset=0,
                            ap=[[K, P], [P * K, NDO], [1, K]])
            nc.sync.dma_start(out=strided(oc_all, 0, [[K, NDO], [1, K]]),
                              in_=ocsrc)
            stage_a(0)
            for c in range(NCHUNK):
                if c + 1 < NCHUNK:
                    stage_a(c + 1)
                if c == 0:
                    late_consts()
                stage_z(c)
                if c >= 1:
                    stage_srep(c)
                if Y_BEFORE_SCAN:
                    if c >= 2:
                        stage_y_tt(c - 1)
                    stage_scan(c)
                    if c >= 2:
                        stage_y_ln(c - 1)
                else:
                    stage_scan(c)
                    if c >= 2:
                        stage_y_tt(c - 1)
                        stage_y_ln(c - 1)
                if c + 1 < NCHUNK:
                    stage_inject(c + 1)
            # final chunk's y-path in quarters, software-pipelined: block
            # q's LN/epilogue chain issues after block q+1's DVE work so
            # the DVE never stalls on the cross-engine LN chain
            stage_y_tt(NCHUNK - 1, 0, 1)
            for tt in range(1, NTT):
                stage_y_tt(NCHUNK - 1, tt, tt + 1)
                stage_y_ln(NCHUNK - 1, tt - 1, tt)
            stage_y_ln(NCHUNK - 1, NTT - 1, NTT)

    nc.compile()
    return nc


def _prep_inputs(x, W_i, W_e, W_s, o_param, ln_gamma, ln_beta, W_out):
    # stable logsigmoid: log sigmoid(w) = min(w,0) - log1p(exp(-|w|))
    o = np.exp(np.log1p(np.exp(-np.abs(o_param))) * (-1.0 / TAU)
               + np.minimum(o_param, 0.0) / TAU).astype(np.float32)
    ocT = np.ascontiguousarray(o.T)                       # (D, K)
    wes = np.concatenate([W_e, W_s], axis=1)
    wo_bf = (ln_gamma[:, None] * W_out).astype(ml_dtypes.bfloat16)
    hrow = wo_bf.astype(np.float32).sum(axis=0, keepdims=True)
    bowr = (ln_beta @ W_out).astype(np.float32)[None, :]
    shared = {
        "wi": np.ascontiguousarray(W_i.astype(ml_dtypes.bfloat16)),
        "wes": np.ascontiguousarray(wes.astype(ml_dtypes.bfloat16)),
        "oc": ocT,
        "wo": np.ascontiguousarray(wo_bf),
        "hrow": np.ascontiguousarray(hrow.astype(ml_dtypes.bfloat16)),
        "bowr": np.ascontiguousarray(bowr),
    }
    if not DPAT_ON_DEVICE:
        dpa = np.zeros((D, K * SEG), np.float32)
        for k in range(K):
            dpa[:, k * SEG + 1:(k + 1) * SEG] = ocT[:, k:k + 1]
        shared["dpa"] = np.ascontiguousarray(dpa)
    in_maps = []
    for core in range(8):
        b, h = core // 2, core % 2
        t0 = h * H
        lo = t0 - W
        if lo < 0:
            xs = np.concatenate(
                [np.zeros((W, D), np.float32), x[b, 0:t0 + H]], axis=0)
        else:
            xs = x[b, lo:t0 + H]
        m = dict(shared)
        m["xt"] = np.ascontiguousarray(xs.T.astype(ml_dtypes.bfloat16))
        in_maps.append(m)
    return in_maps


def kernel(x, W_i, W_e, W_s, o_param, ln_gamma, ln_beta, W_out):
    from concourse.bass_utils import run_bass_kernel_spmd

    global HAS_BOW
    bowr = np.asarray(ln_beta, np.float32) @ np.asarray(W_out, np.float32)
    HAS_BOW = bool(np.any(bowr != 0.0))

    key = ("nc", HAS_BOW)
    if key not in _CACHE:
        _CACHE[key] = _build()
    nc = _CACHE[key]
    _CACHE["nc"] = nc   # test.py reaches in for the TimelineSim estimate

    in_maps = _prep_inputs(np.asarray(x, np.float32), np.asarray(W_i),
                           np.asarray(W_e), np.asarray(W_s),
                           np.asarray(o_param), np.asarray(ln_gamma),
                           np.asarray(ln_beta), np.asarray(W_out))
    res = run_bass_kernel_spmd(nc, in_maps, core_ids=list(range(8)))
    out = np.empty((B, N, D), np.float32)
    for core in range(8):
        b, h = core // 2, core % 2
        out[b, h * H:(h + 1) * H] = res.results[core]["yout"]
    return out


# revision 57
# speedup vs baseline: 1.0025x; 1.0025x over previous
"""Trainium2 Bass kernel for nn_EosLayer (gated linear-attention recurrence).

Sharding: 8 cores = 4 batches x 2 sequence halves. Each core processes
T = 256 (warmup) + 2048 (output) timesteps of one batch half. The warmup
window replaces cross-core state passing: the per-(k,d) decay o < 0.97
makes history older than 256 steps contribute < 1e-3 relative.

Per-core layout is d-major (d on partitions, time on the free dim).
The hardware prefix scan (TensorScalarPtr, DVE-only on TRN2) runs in one
instruction per (chunk, d-tile): the 8 k-slots are fused into a single
scan via an on-device decay pattern with zeroed "break" columns, into
which the previous chunk's carries are injected; the scan state is fp32
internally regardless of operand dtype. The remaining elementwise work
(z = e*i, s*m, k-reduction tree) runs in bf16 (DVE 2x_1p fast mode),
with the z-multiplies offloaded to the Pool/GpSimd engine so the DVE
stays scan+y-path bound. LayerNorm stats come out of the tensor engine
in column form (free-size-1 matmuls), so the LN scalar math is [128, 4]-
shaped and no rsig row->column transpose is needed; gamma/beta are
folded into W_out on the host (the beta term is skipped when it is
exactly zero). The chunk loop is software-pipelined: projections for
chunk c+1 and the y-path for chunk c-1 overlap the scan of chunk c, and
the final chunk's y-path is issued in quarter-width blocks to overlap
its epilogue chain.
"""

import numpy as np
import ml_dtypes

D = 512
K = 8
TAU = 16.0
EPS = 1e-5
B = 4
N = 4096
H = N // 2          # output rows per core
W = 256             # warmup rows
T = W + H           # 2304 rows processed per core
TC = 512            # chunk length (free-dim columns per chunk)
SEG = TC + 1        # k-run length in the scan buffer (break col + TC)
NCHUNK = 5          # chunk 0 is the (256-col) warmup chunk
NDO = 4             # d-tiles of 128 partitions
P = 128
NTT = TC // P       # 4 t-blocks per chunk in the epilogue

_CACHE = {}

# engine assignment knobs (per d-tile, True = Pool/gpsimd, False = DVE).
# TensorScalarPtr (scan, scalar_tensor_tensor) is DVE-only on real HW;
# TensorTensor runs on either engine (Pool at 0.42 efficiency).
Z_POOL = (True, True, True, True)       # z = e*i, chunks 2+
Z1_POOL = (True, False, False, False)   # z for chunk 1 (pipeline ramp)
ZW_POOL = (False, False, True, True)    # warm-chunk z
DPAT_ON_DEVICE = True    # build the decay pattern with Act instead of DMA
SMUL_POOL = (False, False, False, False)
L1_POOL = (False, False, False, False)
L2_POOL = (False, False, False, False)
L3_POOL = (False, False, False, False)
EPI_ACT_POOL = True   # True: Act scale-evac + Pool f32 add; False: DVE stt
Y_BEFORE_SCAN = True   # issue y(c-1) DVE ops before scan(c) in the loop
HAS_BOW = True        # False when beta @ W_out == 0: skip the epilogue add


def _build():
    import concourse.bass as bass
    import concourse.mybir as mybir
    import concourse.tile as tile
    from concourse.bacc import Bacc

    f32 = mybir.dt.float32
    bf16 = mybir.dt.bfloat16
    AF = mybir.ActivationFunctionType
    OP = mybir.AluOpType

    nc = Bacc("TRN2", target_bir_lowering=False, debug=False,
              enable_asserts=False, num_devices=8)

    xt = nc.dram_tensor("xt", (D, T), bf16, kind="ExternalInput")
    wi = nc.dram_tensor("wi", (D, D), bf16, kind="ExternalInput")
    wes = nc.dram_tensor("wes", (D, 2 * K), bf16, kind="ExternalInput")
    oc = nc.dram_tensor("oc", (D, K), f32, kind="ExternalInput")       # o.T
    dpa = None
    if not DPAT_ON_DEVICE:
        dpa = nc.dram_tensor("dpa", (D, K * SEG), f32, kind="ExternalInput")
    wo = nc.dram_tensor("wo", (D, D), bf16, kind="ExternalInput")      # gamma-folded
    hrow = nc.dram_tensor("hrow", (1, D), bf16, kind="ExternalInput")  # colsum(wo)
    bowr = nc.dram_tensor("bowr", (1, D), f32, kind="ExternalInput")   # beta @ W_out
    yout = nc.dram_tensor("yout", (H, D), f32, kind="ExternalOutput")

    def strided(ap, off, dims):
        return bass.AP(tensor=ap.tensor, offset=ap.offset + off,
                       ap=[ap.ap[0]] + dims)

    with tile.TileContext(nc) as tc:
        with tc.tile_pool(name="const", bufs=1) as cst, \
             tc.tile_pool(name="wk", bufs=1) as wk, \
             tc.tile_pool(name="db", bufs=2) as db, \
             tc.tile_pool(name="zmp", bufs=2) as zmp, \
             tc.tile_pool(name="pit", bufs=3, space="PSUM") as pit, \
             tc.tile_pool(name="pes", bufs=1, space="PSUM") as pes, \
             tc.tile_pool(name="pg", bufs=2, space="PSUM") as pg, \
             tc.tile_pool(name="ptc", bufs=1, space="PSUM") as ptc, \
             tc.tile_pool(name="pmr", bufs=1, space="PSUM") as pmr, \
             tc.tile_pool(name="dr", bufs=2, space="DRAM") as dr:

            # ---- constants (loaded once) ----
            wi_all = cst.tile([P, NDO * D], bf16, tag="wi", name="wi")
            wi_sb = [wi_all[:, t * D:(t + 1) * D] for t in range(NDO)]
            wes_all = cst.tile([P, NDO * 2 * K], bf16, tag="wes", name="wes")
            wes_sb = [wes_all[:, t * 2 * K:(t + 1) * 2 * K]
                      for t in range(NDO)]
            oc_all = cst.tile([P, NDO * K], f32, tag="oc", name="oc")
            oc_sb = [oc_all[:, t * K:(t + 1) * K] for t in range(NDO)]
            dp_sb = [cst.tile([P, K * SEG], f32, tag=f"dp{t}", name=f"dp{t}")
                     for t in range(NDO)]
            wo_all = cst.tile([P, NDO * D], bf16, tag="wo", name="wo")
            wo_sb = [wo_all[:, t * D:(t + 1) * D] for t in range(NDO)]
            h_sb = cst.tile([1, D], bf16, tag="h", name="h")
            bow_rep = (cst.tile([P, D], f32, tag="bow", name="bow")
                       if HAS_BOW else None)
            ones_sb = cst.tile([P, 1], bf16, tag="ones", name="ones")
            nc.vector.memset(ones_sb, 1.0)
            eps_col = cst.tile([P, 1], f32, tag="eps", name="eps")
            nc.vector.memset(eps_col, EPS)

            def late_consts():
                # issued after chunk 0's input chain: dpat is first needed
                # by scan(1), wo/h/bow by the chunk-1 epilogue
                if DPAT_ON_DEVICE:
                    # dpat[d, k*SEG] = 0; dpat[d, k*SEG+1..] = oc[d, k]
                    for t in range(NDO):
                        brk = strided(dp_sb[t], 0, [[SEG, K]])
                        nc.gpsimd.memset(brk, 0.0)
                        for k in range(K):
                            nc.scalar.activation(
                                out=dp_sb[t][:, k * SEG + 1:(k + 1) * SEG],
                                in_=wi_sb[t][:, 0:TC], func=AF.Identity,
                                scale=0.0, bias=oc_sb[t][:, k:k + 1])
                else:
                    for t in range(NDO):
                        sl = slice(t * P, (t + 1) * P)
                        nc.sync.dma_start(out=dp_sb[t], in_=dpa[sl, :])
                wosrc = bass.AP(tensor=wo, offset=0,
                                ap=[[D, P], [P * D, NDO], [1, D]])
                nc.sync.dma_start(out=strided(wo_all, 0, [[D, NDO], [1, D]]),
                                  in_=wosrc)
                nc.sync.dma_start(out=h_sb, in_=hrow[:, :])
                if HAS_BOW:
                    bsrc = bass.AP(tensor=bowr, offset=0, ap=[[0, P], [1, D]])
                    nc.sync.dma_start(out=bow_rep, in_=bsrc)

            # ---- per-chunk working tiles (ping-pong via tags) ----
            def cw(c):          # column width of chunk c
                return W if c == 0 else TC

            def c0(c):          # first xt column of chunk c
                return 0 if c == 0 else W + (c - 1) * TC

            xt_sb = {}
            it_sb = {}
            es_sb = {}
            es_d = {}
            e_rep = {}
            s_rep = {}
            zm = {}
            y2t = {}

            def ensure_zm(c):
                if c not in zm:
                    zm[c] = [zmp.tile([P, K * SEG], bf16, tag=f"zm{t}",
                                      name=f"zm{t}") for t in range(NDO)]

            def stage_xt(c):
                w = cw(c)
                xta = db.tile([P, NDO * TC], bf16, tag="xta", name="xta")
                xt_sb[c] = [xta[:, t * TC:(t + 1) * TC] for t in range(NDO)]
                xsrc = bass.AP(tensor=xt, offset=c0(c),
                               ap=[[T, P], [P * T, NDO], [1, w]])
                nc.sync.dma_start(out=strided(xta, 0, [[TC, NDO], [1, w]]),
                                  in_=xsrc)

            def stage_a(c):
                """Projections + e-broadcast for chunk c."""
                w = cw(c)
                if c not in xt_sb:
                    stage_xt(c)
                # e/s projection FIRST: it heads the longest chain
                # (evac -> DRAM bounce -> replicate -> z)
                es_ps = pes.tile([2 * K, TC], f32, tag="esps", name="esps")
                for kt in range(NDO):
                    nc.tensor.matmul(es_ps[:, 0:w], wes_sb[kt][:, :],
                                     xt_sb[c][kt][:, 0:w],
                                     start=(kt == 0), stop=(kt == NDO - 1))
                es_sb[c] = wk.tile([2 * K, TC], bf16, tag="es", name="es")
                nc.scalar.copy(out=es_sb[c][:, 0:w], in_=es_ps[:, 0:w])
                es_d[c] = dr.tile([2 * K, TC], bf16, tag="esd", name="esd")
                nc.sync.dma_start(out=es_d[c][:, 0:w], in_=es_sb[c][:, 0:w])
                # replicate e rows across all partitions (via DRAM bounce);
                # chunk 0 in two k-halves so the first warm z starts as soon
                # as half the broadcast lands (startup critical path)
                e_rep[c] = db.tile([P, K * TC], bf16, tag="erep", name="erep")
                nh = 2 if c == 0 else 1
                kh = K // nh
                for h in range(nh):
                    esrc = bass.AP(tensor=es_d[c].tensor,
                                   offset=es_d[c].offset + h * kh * TC,
                                   ap=[[0, P], [TC, kh], [1, w]])
                    nc.sync.dma_start(
                        out=strided(e_rep[c], h * kh * w, [[w, kh], [1, w]]),
                        in_=esrc)
                stage_ai(c)

            def stage_ai(c):
                # i projection: 4 m-tiles x 4 contraction tiles
                w = cw(c)
                it_sb[c] = [db.tile([P, TC], bf16, tag=f"it{t}", name=f"it{t}")
                            for t in range(NDO)]
                for m in range(NDO):
                    it_ps = pit.tile([P, TC], f32, tag="itps", name="itps")
                    for kt in range(NDO):
                        nc.tensor.matmul(
                            it_ps[:, 0:w],
                            wi_sb[kt][:, m * P:(m + 1) * P],
                            xt_sb[c][kt][:, 0:w],
                            start=(kt == 0), stop=(kt == NDO - 1))
                    nc.scalar.copy(out=it_sb[c][m][:, 0:w], in_=it_ps[:, 0:w])

            def stage_srep(c):
                """Replicate s rows (consumed by stage_y(c) one chunk later)."""
                s_rep[c] = db.tile([P, K * TC], bf16, tag="srep", name="srep")
                ssrc = bass.AP(tensor=es_d[c].tensor,
                               offset=es_d[c].offset + K * TC,
                               ap=[[0, P], [TC, K], [1, TC]])
                nc.sync.dma_start(out=s_rep[c], in_=ssrc)

            def stage_z(c):
                """z = e * i into the scan buffer (bf16 2x on DVE)."""
                w = cw(c)
                seg = w + (0 if c == 0 else 1)
                off = 0 if c == 0 else 1
                ensure_zm(c)
                pool_map = (ZW_POOL if c == 0 else
                            Z1_POOL if c == 1 else Z_POOL)
                nh = 2 if c == 0 else 1
                kh = K // nh
                for t in range(NDO):
                    eng = nc.gpsimd if pool_map[t] else nc.vector
                    for h in range(nh):
                        zv = strided(zm[c][t], off + h * kh * seg,
                                     [[seg, kh], [1, w]])
                        ev = strided(e_rep[c], h * kh * w, [[w, kh], [1, w]])
                        iv = strided(it_sb[c][t], 0, [[0, kh], [1, w]])
                        eng.tensor_mul(out=zv, in0=ev, in1=iv)

            def stage_scan(c):
                """Prefix scan over time, all 8 k fused (Pool)."""
                w = cw(c)
                for t in range(NDO):
                    if c == 0:
                        # per-k scans, no carry, decay broadcast from oc col
                        for k in range(K):
                            col = oc_sb[t][:, k:k + 1]
                            dec = bass.AP(tensor=col.tensor, offset=col.offset,
                                          ap=[col.ap[0], [0, w]])
                            seg = zm[c][t][:, k * w:(k + 1) * w]
                            nc.vector.tensor_tensor_scan(
                                out=seg, data0=dec, data1=seg,
                                initial=0.0, op0=OP.mult, op1=OP.add)
                    else:
                        buf = zm[c][t][:, 0:K * SEG]
                        nc.vector.tensor_tensor_scan(
                            out=buf, data0=dp_sb[t][:, :], data1=buf,
                            initial=0.0, op0=OP.mult, op1=OP.add)

            def stage_inject(c):
                """Carry (break) columns for chunk c from chunk c-1's output."""
                pw = cw(c - 1)
                pseg = pw + (0 if c == 1 else 1)
                poff = (pw - 1) if c == 1 else pw
                ensure_zm(c)
                for t in range(NDO):
                    src = strided(zm[c - 1][t], poff, [[pseg, K]])
                    dst = strided(zm[c][t], 0, [[SEG, K]])
                    nc.scalar.copy(out=dst, in_=src)

            ytile = {}

            def stage_y_tt(c, lo=0, hi=NTT):
                """s*m and k-reduction for chunk c, t-blocks [lo, hi)."""
                b0 = lo * P
                w = (hi - lo) * P
                if lo == 0:
                    ytile[c] = (
                        [wk.tile([P, 2 * TC], bf16, tag=f"y2t{t}",
                                 name=f"y2t{t}") for t in range(NDO)],
                        ptc.tile([P, 2 * NTT], f32, tag="tcol", name="tcol"),
                        pmr.tile([1, TC], f32, tag="mrow", name="mrow"),
                        wk.tile([1, TC], bf16, tag="mneg", name="mneg"),
                        wk.tile([P, NTT], f32, tag="mu2c", name="mu2c"),
                        wk.tile([P, NTT], f32, tag="varc", name="varc"),
                        wk.tile([P, NTT], f32, tag="sigc", name="sigc"),
                        wk.tile([P, NTT], f32, tag="rsig", name="rsig"),
                    )
                    y2t[c] = ytile[c][0]
                yts, tcol, mrow, mneg, mu2c, varc, sigc, rsig = ytile[c]
                for t in range(NDO):
                    mv = strided(zm[c][t], 1 + b0, [[SEG, K], [1, w]])
                    sv = strided(s_rep[c], b0, [[TC, K], [1, w]])
                    e1 = nc.gpsimd if SMUL_POOL[t] else nc.vector
                    e1.tensor_mul(out=mv, in0=mv, in1=sv)
                    lo1 = strided(zm[c][t], 1 + b0, [[SEG, K // 2], [1, w]])
                    hi1 = strided(zm[c][t], 1 + b0 + (K // 2) * SEG,
                                  [[SEG, K // 2], [1, w]])
                    e2 = nc.gpsimd if L1_POOL[t] else nc.vector
                    e2.tensor_add(out=lo1, in0=lo1, in1=hi1)
                    lo2 = strided(zm[c][t], 1 + b0, [[SEG, 2], [1, w]])
                    hi2 = strided(zm[c][t], 1 + b0 + 2 * SEG, [[SEG, 2], [1, w]])
                    e3 = nc.gpsimd if L2_POOL[t] else nc.vector
                    e3.tensor_add(out=lo2, in0=lo2, in1=hi2)
                    e4 = nc.gpsimd if L3_POOL[t] else nc.vector
                    e4.tensor_add(
                        out=yts[t][:, b0:b0 + w],
                        in0=zm[c][t][:, 1 + b0:1 + b0 + w],
                        in1=zm[c][t][:, 1 + SEG + b0:1 + SEG + b0 + w])
                    nc.scalar.activation(out=yts[t][:, TC + b0:TC + b0 + w],
                                         in_=yts[t][:, b0:b0 + w],
                                         func=AF.Square, scale=1.0)

            def stage_y_ln(c, lo=0, hi=NTT):
                """LN stats + epilogue for chunk c, t-blocks [lo, hi)."""
                b0 = lo * P
                w = (hi - lo) * P
                yts, tcol, mrow, mneg, mu2c, varc, sigc, rsig = ytile[c]
                # LN stats in column form: free-size-1 matmuls
                for tt in range(lo, hi):
                    tsl = slice(tt * P, (tt + 1) * P)
                    for t in range(NDO):
                        nc.tensor.matmul(tcol[:, tt:tt + 1],
                                         yts[t][:, tsl], ones_sb[:, :],
                                         start=(t == 0), stop=(t == NDO - 1))
                    for t in range(NDO):
                        nc.tensor.matmul(
                            tcol[:, NTT + tt:NTT + tt + 1],
                            yts[t][:, TC + tt * P:TC + (tt + 1) * P],
                            ones_sb[:, :],
                            start=(t == 0), stop=(t == NDO - 1))
                # row form of the mean (for the rank-1 G correction)
                for t in range(NDO):
                    nc.tensor.matmul(mrow[:, b0:b0 + w], ones_sb[:, :],
                                     yts[t][:, b0:b0 + w],
                                     start=(t == 0), stop=(t == NDO - 1))
                nc.scalar.activation(out=mneg[:, b0:b0 + w],
                                     in_=mrow[:, b0:b0 + w],
                                     func=AF.Copy, scale=-1.0 / D)
                # rsig columns: 1/sqrt(Q/D - (M/D)^2 + eps), all [P, hi-lo]
                nc.scalar.activation(out=mu2c[:, lo:hi], in_=tcol[:, lo:hi],
                                     func=AF.Square, scale=1.0 / D)
                nc.vector.scalar_tensor_tensor(
                    out=varc[:, lo:hi], in0=tcol[:, NTT + lo:NTT + hi],
                    scalar=1.0 / D, in1=mu2c[:, lo:hi],
                    op0=OP.mult, op1=OP.subtract)
                nc.scalar.activation(out=sigc[:, lo:hi], in_=varc[:, lo:hi],
                                     func=AF.Sqrt, bias=eps_col[:, :],
                                     scale=1.0)
                nc.vector.reciprocal(out=rsig[:, lo:hi], in_=sigc[:, lo:hi])

                # epilogue: G = y^T Wo' + mneg x h; out = G*rsig + beta@Wout
                for tt in range(lo, hi):
                    tsl = slice(tt * P, (tt + 1) * P)
                    g_ps = pg.tile([P, D], f32, tag="gps", name="gps")
                    for t in range(NDO):
                        nc.tensor.matmul(g_ps[:, :], yts[t][:, tsl],
                                         wo_sb[t][:, :],
                                         start=(t == 0), stop=False)
                    nc.tensor.matmul(g_ps[:, :], mneg[:, tsl], h_sb[:, :],
                                     start=False, stop=True)
                    out_sb = db.tile([P, D], f32, tag="outp", name="outp")
                    if not HAS_BOW:
                        nc.scalar.activation(out=out_sb[:, :], in_=g_ps[:, :],
                                             func=AF.Copy,
                                             scale=rsig[:, tt:tt + 1])
                    elif EPI_ACT_POOL:
                        gs = db.tile([P, D], f32, tag="gsc", name="gsc")
                        nc.scalar.activation(out=gs[:, :], in_=g_ps[:, :],
                                             func=AF.Copy,
                                             scale=rsig[:, tt:tt + 1])
                        nc.gpsimd.tensor_add(out=out_sb[:, :], in0=gs[:, :],
                                             in1=bow_rep[:, :])
                    else:
                        nc.vector.scalar_tensor_tensor(
                            out=out_sb[:, :], in0=g_ps[:, :],
                            scalar=rsig[:, tt:tt + 1], in1=bow_rep[:, :],
                            op0=OP.mult, op1=OP.add)
                    orow = (c - 1) * TC + tt * P
                    nc.sync.dma_start(out=yout[orow:orow + P, :],
                                      in_=out_sb[:, :])

            # ---- software-pipelined chunk loop ----
            # iter c issues: proj(c+1) | z(c) | scan(c) | y(c-1) | carries(c+1)
            # chunk-0 inputs lead the DMA queue; then the critical consts
            stage_xt(0)
            wessrc = bass.AP(tensor=wes, offset=0,
                             ap=[[2 * K, P], [P * 2 * K, NDO], [1, 2 * K]])
            nc.sync.dma_start(
                out=strided(wes_all, 0, [[2 * K, NDO], [1, 2 * K]]),
                in_=wessrc)
            wisrc = bass.AP(tensor=wi, offset=0,
                            ap=[[D, P], [P * D, NDO], [1, D]])
            nc.sync.dma_start(out=strided(wi_all, 0, [[D, NDO], [1, D]]),
                              in_=wisrc)
            ocsrc = bass.AP(tensor=oc, offset=0,
                            ap=[[K, P], [P * K, NDO], [1, K]])
            nc.sync.dma_start(out=strided(oc_all, 0, [[K, NDO], [1, K]]),
                              in_=ocsrc)
            stage_a(0)
            for c in range(NCHUNK):
                if c + 1 < NCHUNK:
                    stage_a(c + 1)
                if c == 0:
                    late_consts()
                stage_z(c)
                if c >= 1:
                    stage_srep(c)
                if Y_BEFORE_SCAN:
                    if c >= 2:
                        stage_y_tt(c - 1)
                    stage_scan(c)
                    if c >= 2:
                        stage_y_ln(c - 1)
                else:
                    stage_scan(c)
                    if c >= 2:
                        stage_y_tt(c - 1)
                        stage_y_ln(c - 1)
                if c + 1 < NCHUNK:
                    stage_inject(c + 1)
            # final chunk's y-path in quarters, software-pipelined: block
            # q's LN/epilogue chain issues after block q+1's DVE work so
            # the DVE never stalls on the cross-engine LN chain
            stage_y_tt(NCHUNK - 1, 0, 1)
            for tt in range(1, NTT):
                stage_y_tt(NCHUNK - 1, tt, tt + 1)
                stage_y_ln(NCHUNK - 1, tt - 1, tt)
            stage_y_ln(NCHUNK - 1, NTT - 1, NTT)

    nc.compile()
    return nc


def _prep_inputs(x, W_i, W_e, W_s, o_param, ln_gamma, ln_beta, W_out):
    # stable logsigmoid: log sigmoid(w) = min(w,0) - log1p(exp(-|w|))
    o = np.exp(np.log1p(np.exp(-np.abs(o_param))) * (-1.0 / TAU)
               + np.minimum(o_param, 0.0) / TAU).astype(np.float32)
    ocT = np.ascontiguousarray(o.T)                       # (D, K)
    wes = np.concatenate([W_e, W_s], axis=1)
    wo_bf = (ln_gamma[:, None] * W_out).astype(ml_dtypes.bfloat16)
    hrow = wo_bf.astype(np.float32).sum(axis=0, keepdims=True)
    bowr = (ln_beta @ W_out).astype(np.float32)[None, :]
    shared = {
        "wi": np.ascontiguousarray(W_i.astype(ml_dtypes.bfloat16)),
        "wes": np.ascontiguousarray(wes.astype(ml_dtypes.bfloat16)),
        "oc": ocT,
        "wo": np.ascontiguousarray(wo_bf),
        "hrow": np.ascontiguousarray(hrow.astype(ml_dtypes.bfloat16)),
        "bowr": np.ascontiguousarray(bowr),
    }
    if not DPAT_ON_DEVICE:
        dpa = np.zeros((D, K * SEG), np.float32)
        for k in range(K):
            dpa[:, k * SEG + 1:(k + 1) * SEG] = ocT[:, k:k + 1]
        shared["dpa"] = np.ascontiguousarray(dpa)
    in_maps = []
    for core in range(8):
        b, h = core // 2, core % 2
        t0 = h * H
        lo = t0 - W
        if lo < 0:
            xs = np.concatenate(
                [np.zeros((W, D), np.float32), x[b, 0:t0 + H]], axis=0)
        else:
            xs = x[b, lo:t0 + H]
        m = dict(shared)
        m["xt"] = np.ascontiguousarray(xs.T.astype(ml_dtypes.bfloat16))
        in_maps.append(m)
    return in_maps


def kernel(x, W_i, W_e, W_s, o_param, ln_gamma, ln_beta, W_out):
    from concourse.bass_utils import run_bass_kernel_spmd

    global HAS_BOW
    bowr = np.asarray(ln_beta, np.float32) @ np.asarray(W_out, np.float32)
    HAS_BOW = bool(np.any(bowr != 0.0))

    key = ("nc", HAS_BOW)
    if key not in _CACHE:
        _CACHE[key] = _build()
    nc = _CACHE[key]
    _CACHE["nc"] = nc   # test.py reaches in for the TimelineSim estimate

    in_maps = _prep_inputs(np.asarray(x, np.float32), np.asarray(W_i),
                           np.asarray(W_e), np.asarray(W_s),
                           np.asarray(o_param), np.asarray(ln_gamma),
                           np.asarray(ln_beta), np.asarray(W_out))
    res = run_bass_kernel_spmd(nc, in_maps, core_ids=list(range(8)))
    out = np.empty((B, N, D), np.float32)
    for core in range(8):
        b, h = core // 2, core % 2
        out[b, h * H:(h + 1) * H] = res.results[core]["yout"]
    return out
